# revision 1
# baseline (speedup 1.0000x reference)
"""GAT message-passing kernel for Trainium2 (8 NeuronCores, SPMD).

Strategy: shard edges by TARGET node range (each core owns NLOC=6272 of the
padded 50176 nodes and all edges targeting them). Per core, targets are
grouped into 49 tiles of 128 nodes; each tile's edges are processed in
chunks of 128:
  - per-edge x[src], sj[src]: batched dma_gather (int16 idx; lo/hi arena
    split at 32768 source rows)
  - per-edge si[tgt], recip[tgt]: batched dma_gather from a core-local
    [si|recip] table (tgt_local < 6272 fits int16 directly)
  - denominator segment-sum and output scatter-add: one-hot matmuls
    (one-hot built by DVE iota==tgt_off compare) accumulated in PSUM
Score tables si/sj are computed once from each core's node slice; sj is
AllGather'd. No other collectives are needed.
"""
import numpy as np

import concourse.mybir as mybir
from concourse import bacc, bass_utils
from concourse.tile import TileContext

P = 128
NCORES = 8
N_NODES = 50000
N_EDGES = 800000
HID = 128
HEADS = 8
NPAD = 50176              # 8 * 6272
NLOC = NPAD // NCORES     # 6272 nodes per core
NT = NLOC // P            # 49 tiles per core
SPLIT = 32768             # lo/hi arena split for int16 dma_gather indices
NEG_SLOPE = 0.01
SJW = 64                  # sj table row width (f32) -> 256B rows for dma_gather
SRW = 64                  # [si|recip] table row width
GMAX = 8                  # slots per dma_gather call (1024 idx HW limit)

_CACHE = {}


def _build_program(nclo, nchi, batches):
    nch = [lo + hi for lo, hi in zip(nclo, nchi)]
    nchunks = sum(nch)
    nslot_lo = sum(nclo) * P
    nslot_hi = sum(nchi) * P
    lo_base = np.cumsum([0] + nclo).tolist()
    hi_base = np.cumsum([0] + nchi).tolist()
    ch_base = np.cumsum([0] + nch).tolist()

    nc = bacc.Bacc("TRN2", num_devices=NCORES)
    f32 = mybir.dt.float32

    xpad = nc.dram_tensor("xpad", [NPAD, HID], f32, kind="ExternalInput")
    xslice = nc.dram_tensor("xslice", [NLOC, HID], f32, kind="ExternalInput")
    wcat = nc.dram_tensor("wcat", [HID, 2 * HEADS], f32, kind="ExternalInput")
    idxlo = nc.dram_tensor("idxlo", [P, max(nslot_lo // 16, 1)],
                           mybir.dt.int16, kind="ExternalInput")
    idxhi = nc.dram_tensor("idxhi", [P, max(nslot_hi // 16, 1)],
                           mybir.dt.int16, kind="ExternalInput")
    idxtg = nc.dram_tensor("idxtg", [P, nchunks * 8], mybir.dt.int16,
                           kind="ExternalInput")
    toffin = nc.dram_tensor("toffin", [P, nchunks], f32, kind="ExternalInput")
    out_sl = nc.dram_tensor("out_sl", [NLOC, HID], f32, kind="ExternalOutput")

    sjtab = nc.dram_tensor("sjtab", [NPAD, SJW], f32, kind="Internal")
    sitab = nc.dram_tensor("sitab", [NLOC, SRW], f32, kind="Internal")
    retab = nc.dram_tensor("retab", [NLOC, SRW], f32, kind="Internal")
    cc_in = nc.dram_tensor("cc_in", [NLOC, HEADS], f32, kind="Internal")
    cc_out = nc.dram_tensor("cc_out", [NPAD, HEADS], f32, kind="Internal",
                            addr_space="Shared")

    ident_d = nc.inline_tensor(np.eye(P, dtype=np.float32), name="identc")
    iota_d = nc.inline_tensor(
        np.tile(np.arange(P, dtype=np.float32), (P, 1)), name="iotac")
    zero_d = nc.inline_tensor(np.zeros((P, SJW), np.float32), name="zeroc")

    with TileContext(nc) as tc:
        with tc.tile_pool(name="const", bufs=1) as constp, \
             tc.tile_pool(name="ph0", bufs=3) as ph0:

            ident = constp.tile([P, P], f32)
            nc.sync.dma_start(out=ident[:], in_=ident_d[:, :])
            iota_f = constp.tile([P, P], f32)
            nc.sync.dma_start(out=iota_f[:], in_=iota_d[:, :])
            wc_sb = constp.tile([HID, 2 * HEADS], f32)
            nc.sync.dma_start(out=wc_sb[:], in_=wcat[:, :])
            toff_sb = constp.tile([P, nchunks], f32)
            nc.sync.dma_start(out=toff_sb[:], in_=toffin[:, :])
            ixlo_sb = constp.tile([P, max(nslot_lo // 16, 1)], mybir.dt.int16)
            nc.sync.dma_start(out=ixlo_sb[:], in_=idxlo[:, :])
            ixhi_sb = constp.tile([P, max(nslot_hi // 16, 1)], mybir.dt.int16)
            nc.sync.dma_start(out=ixhi_sb[:], in_=idxhi[:, :])
            ixtg_sb = constp.tile([P, nchunks * 8], mybir.dt.int16)
            nc.sync.dma_start(out=ixtg_sb[:], in_=idxtg[:, :])
            zt = constp.tile([P, 1, SJW], f32)
            nc.sync.dma_start(out=zt[:, 0, :], in_=zero_d[:, :])

            # zero pad columns of both gather tables (gathers read whole
            # 256B rows; sim rejects uninitialized reads)
            nc.sync.dma_start(
                out=sjtab[:, HEADS:SJW].rearrange("(t p) w -> p t w", p=P),
                in_=zt[:, :, 0:SJW - HEADS].to_broadcast(
                    [P, NPAD // P, SJW - HEADS]))
            nc.sync.dma_start(
                out=sitab[:, HEADS:SRW].rearrange("(t p) w -> p t w", p=P),
                in_=zt[:, :, 0:SRW - HEADS].to_broadcast(
                    [P, NT, SRW - HEADS]))
            nc.sync.dma_start(
                out=retab[:, 0:SRW].rearrange("(t p) w -> p t w", p=P),
                in_=zt[:, :, 0:SRW].to_broadcast([P, NT, SRW]))

            # ---------- phase 0: score tables ----------
            with tc.tile_pool(name="ph0ps", bufs=2, space="PSUM") as ph0ps:
                for j in range(NT):
                    xt = ph0.tile([P, HID], f32, tag="xt")
                    nc.sync.dma_start(out=xt[:], in_=xslice[j * P:(j + 1) * P, :])
                    xT_ps = ph0ps.tile([P, P], f32, space="PSUM", tag="xTp")
                    nc.tensor.transpose(out=xT_ps[:], in_=xt[:], identity=ident[:])
                    xT = ph0.tile([P, P], f32, tag="xT")
                    nc.scalar.copy(out=xT[:], in_=xT_ps[:])
                    sc_ps = ph0ps.tile([P, 2 * HEADS], f32, space="PSUM", tag="scp")
                    nc.tensor.matmul(out=sc_ps[:], lhsT=xT[:], rhs=wc_sb[:],
                                     start=True, stop=True)
                    sc = ph0.tile([P, 2 * HEADS], f32, tag="sc")
                    nc.vector.tensor_copy(out=sc[:], in_=sc_ps[:])
                    nc.sync.dma_start(out=sitab[j * P:(j + 1) * P, 0:HEADS],
                                      in_=sc[:, 0:HEADS])
                    nc.sync.dma_start(out=cc_in[j * P:(j + 1) * P, :],
                                      in_=sc[:, HEADS:2 * HEADS])

            nc.gpsimd.collective_compute(
                "AllGather", mybir.AluOpType.bypass,
                replica_groups=[list(range(NCORES))],
                ins=[cc_in[:, :]], outs=[cc_out[:, :]],
            )
            nc.sync.dma_start(
                out=sjtab[:, 0:HEADS].rearrange("(t p) w -> p t w", p=P),
                in_=cc_out[:, :].rearrange("(t p) w -> p t w", p=P))

            with tc.tile_pool(name="gat", bufs=2) as gatp, \
                 tc.tile_pool(name="oh", bufs=5) as ohp, \
                 tc.tile_pool(name="tile", bufs=4) as tilep, \
                 tc.tile_pool(name="sm", bufs=4) as smp, \
                 tc.tile_pool(name="ps_den", bufs=4, space="PSUM") as psd, \
                 tc.tile_pool(name="ps_out", bufs=4, space="PSUM") as pso:

                def gcalls(dst, table_ap, idx_sb, s0, s1):
                    ew = dst.shape[-1]
                    for g0 in range(0, s1 - s0, GMAX):
                        g1 = min(g0 + GMAX, s1 - s0)
                        nidx = (g1 - g0) * P
                        nc.gpsimd.dma_gather(
                            out_ap=dst[:, g0:g1, :], in_ap=table_ap,
                            idxs_ap=idx_sb[:, (s0 + g0) * 8:(s0 + g1) * 8],
                            num_idxs=nidx, num_idxs_reg=nidx, elem_size=ew)

                for (t0, t1) in batches:
                    blo0, blo1 = lo_base[t0], lo_base[t1]
                    bhi0, bhi1 = hi_base[t0], hi_base[t1]
                    bch0, bch1 = ch_base[t0], ch_base[t1]

                    gx_lo = gatp.tile([P, max(blo1 - blo0, 1), HID], f32,
                                      tag="gxlo")
                    gs_lo = gatp.tile([P, max(blo1 - blo0, 1), SJW], f32,
                                      tag="gslo")
                    if blo1 > blo0:
                        gcalls(gx_lo[:], xpad[:, :], ixlo_sb, blo0, blo1)
                        gcalls(gs_lo[:], sjtab[:, :], ixlo_sb, blo0, blo1)
                    gx_hi = gatp.tile([P, max(bhi1 - bhi0, 1), HID], f32,
                                      tag="gxhi")
                    gs_hi = gatp.tile([P, max(bhi1 - bhi0, 1), SJW], f32,
                                      tag="gshi")
                    if bhi1 > bhi0:
                        gcalls(gx_hi[:], xpad[SPLIT:NPAD, :], ixhi_sb, bhi0, bhi1)
                        gcalls(gs_hi[:], sjtab[SPLIT:NPAD, :], ixhi_sb, bhi0, bhi1)
                    # si gather (by tgt_local), valid cols 0:8
                    gsa = gatp.tile([P, bch1 - bch0, SRW], f32, tag="gsa")
                    gcalls(gsa[:], sitab[:, :], ixtg_sb, bch0, bch1)

                    # ---------- phase A per tile ----------
                    tile_state = []
                    for j in range(t0, t1):
                        ncj = nch[j]
                        nlo_j, nhi_j = nclo[j], nchi[j]
                        ch0 = ch_base[j]
                        ohs = []
                        ex = tilep.tile([P, ncj * HEADS], f32, tag="ex")
                        co = ch0 - bch0
                        if nlo_j:
                            s0 = lo_base[j] - blo0
                            nc.vector.tensor_tensor(
                                out=ex[:, 0:nlo_j * HEADS].rearrange(
                                    "p (k w) -> p k w", k=nlo_j),
                                in0=gsa[:, co:co + nlo_j, 0:HEADS],
                                in1=gs_lo[:, s0:s0 + nlo_j, 0:HEADS],
                                op=mybir.AluOpType.add)
                        if nhi_j:
                            s0 = hi_base[j] - bhi0
                            nc.vector.tensor_tensor(
                                out=ex[:, nlo_j * HEADS:ncj * HEADS].rearrange(
                                    "p (k w) -> p k w", k=nhi_j),
                                in0=gsa[:, co + nlo_j:co + ncj, 0:HEADS],
                                in1=gs_hi[:, s0:s0 + nhi_j, 0:HEADS],
                                op=mybir.AluOpType.add)
                        lk = tilep.tile([P, ncj * HEADS], f32, tag="lk")
                        nc.vector.tensor_scalar(
                            out=lk[:], in0=ex[:], scalar1=NEG_SLOPE,
                            scalar2=None, op0=mybir.AluOpType.mult)
                        nc.vector.tensor_tensor(out=ex[:], in0=ex[:], in1=lk[:],
                                                op=mybir.AluOpType.max)
                        nc.scalar.activation(
                            out=ex[:], in_=ex[:],
                            func=mybir.ActivationFunctionType.Exp)
                        den_ps = psd.tile([P, HEADS], f32, space="PSUM",
                                          tag="denps")
                        for c in range(ncj):
                            oh = ohp.tile([P, P], f32, tag=f"oh{c}")
                            nc.vector.tensor_scalar(
                                out=oh[:], in0=iota_f[:],
                                scalar1=toff_sb[:, ch0 + c:ch0 + c + 1],
                                scalar2=None, op0=mybir.AluOpType.is_equal)
                            nc.tensor.matmul(
                                out=den_ps[:], lhsT=oh[:],
                                rhs=ex[:, c * HEADS:(c + 1) * HEADS],
                                start=(c == 0), stop=(c == ncj - 1))
                            ohs.append(oh)
                        rec = smp.tile([P, HEADS], f32, tag="rec")
                        nc.vector.tensor_scalar(
                            out=rec[:], in0=den_ps[:], scalar1=1e-30,
                            scalar2=None, op0=mybir.AluOpType.max)
                        nc.vector.reciprocal(out=rec[:], in_=rec[:])
                        nc.vector.tensor_scalar(
                            out=rec[:], in0=rec[:], scalar1=1.0 / HEADS,
                            scalar2=None, op0=mybir.AluOpType.mult)
                        nc.sync.dma_start(
                            out=retab[j * P:(j + 1) * P, 0:HEADS],
                            in_=rec[:])
                        tile_state.append((j, ex, ohs))

                    # recip gather for the whole batch (rows now updated)
                    gsb = gatp.tile([P, bch1 - bch0, SRW], f32, tag="gsb")
                    gcalls(gsb[:], retab[:, :], ixtg_sb, bch0, bch1)

                    # ---------- phase B per tile ----------
                    for (j, ex, ohs) in tile_state:
                        ncj = nch[j]
                        nlo_j = nclo[j]
                        ch0 = ch_base[j]
                        co = ch0 - bch0
                        prod = smp.tile([P, ncj * HEADS], f32, tag="prod")
                        nc.vector.tensor_tensor(
                            out=prod[:].rearrange("p (k w) -> p k w", k=ncj),
                            in0=gsb[:, co:co + ncj, 0:HEADS],
                            in1=ex[:].rearrange("p (k w) -> p k w", k=ncj),
                            op=mybir.AluOpType.mult)
                        alpha = smp.tile([P, ncj], f32, tag="alpha")
                        nc.vector.reduce_sum(
                            out=alpha[:],
                            in_=prod[:].rearrange("p (k w) -> p k w", k=ncj),
                            axis=mybir.AxisListType.X)
                        out_ps = pso.tile([P, HID], f32, space="PSUM",
                                          tag="outps")
                        for c in range(ncj):
                            if c < nlo_j:
                                gx_ap = gx_lo[:, lo_base[j] - blo0 + c, :]
                            else:
                                gx_ap = gx_hi[:, hi_base[j] - bhi0 + (c - nlo_j), :]
                            oha = ohs[c]
                            nc.vector.tensor_scalar(
                                out=oha[:], in0=oha[:],
                                scalar1=alpha[:, c:c + 1],
                                scalar2=None, op0=mybir.AluOpType.mult)
                            nc.tensor.matmul(out=out_ps[:], lhsT=oha[:],
                                             rhs=gx_ap,
                                             start=(c == 0), stop=(c == ncj - 1))
                        ot = smp.tile([P, HID], f32, tag="ot")
                        nc.scalar.copy(out=ot[:], in_=out_ps[:])
                        nc.sync.dma_start(out=out_sl[j * P:(j + 1) * P, :],
                                          in_=ot[:])

    nc.compile()
    return nc


def _prep(edge_index):
    """Host-side edge layout -> per-core index/toff arrays + chunk schedule."""
    src = edge_index[0].astype(np.int64)
    tgt = edge_index[1].astype(np.int64)
    core = tgt // NLOC
    tile = (tgt % NLOC) // P
    toff = tgt % P
    tloc = tgt % NLOC
    lo = src < SPLIT

    counts = np.zeros((NCORES, NT, 2), np.int64)
    np.add.at(counts, (core, tile, (~lo).astype(np.int64)), 1)
    nclo = [int(np.ceil(max(counts[:, j, 0].max(), 1) / P)) for j in range(NT)]
    nchi = [int(np.ceil(counts[:, j, 1].max() / P))
            if counts[:, j, 1].max() > 0 else 0 for j in range(NT)]

    nch = [a + b for a, b in zip(nclo, nchi)]
    nchunks = sum(nch)
    nslot_lo = sum(nclo) * P
    nslot_hi = sum(nchi) * P
    lo_base = np.cumsum([0] + nclo)
    hi_base = np.cumsum([0] + nchi)
    ch_base = np.cumsum([0] + nch)

    per_core = []
    order = np.lexsort((tile, core))
    src_s, tile_s, toff_s, lo_s, core_s, tloc_s = (
        src[order], tile[order], toff[order], lo[order], core[order],
        tgt[order] % NLOC)
    cuts = np.searchsorted(core_s, np.arange(NCORES + 1))

    def wrap16(a):
        if len(a) == 0:
            return np.zeros((P, 1), np.int16)
        w = a.reshape(-1, 16).T
        return np.tile(w, (8, 1)).astype(np.int16)

    for c in range(NCORES):
        s, e = cuts[c], cuts[c + 1]
        csrc, ctile, ctoff, clo, ctloc = (src_s[s:e], tile_s[s:e],
                                          toff_s[s:e], lo_s[s:e], tloc_s[s:e])
        ilo = np.zeros(nslot_lo, np.int16)
        ihi = np.zeros(nslot_hi, np.int16)
        itg = np.zeros(nchunks * P, np.int16)
        tof = np.full(nchunks * P, 999.0, np.float32)
        tcuts = np.searchsorted(ctile, np.arange(NT + 1))
        for j in range(NT):
            js, je = tcuts[j], tcuts[j + 1]
            jsrc, jtoff, jlo, jtloc = (csrc[js:je], ctoff[js:je], clo[js:je],
                                       ctloc[js:je])
            sel = jlo
            n = int(sel.sum())
            ilo[lo_base[j] * P:lo_base[j] * P + n] = jsrc[sel].astype(np.int16)
            cb = ch_base[j] * P
            tof[cb:cb + n] = jtoff[sel]
            itg[cb:cb + n] = jtloc[sel].astype(np.int16)
            sel = ~jlo
            m = int(sel.sum())
            ihi[hi_base[j] * P:hi_base[j] * P + m] = \
                (jsrc[sel] - SPLIT).astype(np.int16)
            cb2 = (ch_base[j] + nclo[j]) * P
            tof[cb2:cb2 + m] = jtoff[sel]
            itg[cb2:cb2 + m] = jtloc[sel].astype(np.int16)

        per_core.append({
            "idxlo": wrap16(ilo),
            "idxhi": wrap16(ihi),
            "idxtg": wrap16(itg),
            "toffin": np.ascontiguousarray(
                tof.reshape(nchunks, P).T).astype(np.float32),
        })
    return nclo, nchi, per_core


def _in_maps(inputs, per_core):
    xpad = np.zeros((NPAD, HID), np.float32)
    xpad[:N_NODES] = inputs["x"]
    wcat = np.concatenate([np.asarray(inputs["Wi"]).T,
                           np.asarray(inputs["Wj"]).T],
                          axis=1).astype(np.float32)
    maps = []
    for c in range(NCORES):
        m = dict(per_core[c])
        m["xpad"] = xpad
        m["xslice"] = np.ascontiguousarray(xpad[c * NLOC:(c + 1) * NLOC])
        m["wcat"] = wcat
        maps.append(m)
    return maps


def kernel(x, Wi, Wj, edge_index):
    inputs = {"x": np.asarray(x, np.float32),
              "Wi": np.asarray(Wi, np.float32),
              "Wj": np.asarray(Wj, np.float32)}
    edge_index = np.asarray(edge_index)

    nclo, nchi, per_core = _prep(edge_index)
    key = (tuple(nclo), tuple(nchi))
    if key not in _CACHE:
        batches = [(t, min(t + 2, NT)) for t in range(0, NT, 2)]
        _CACHE.clear()
        _CACHE[key] = _build_program(nclo, nchi, batches)
    nc = _CACHE[key]

    res = bass_utils.run_bass_kernel_spmd(nc, _in_maps(inputs, per_core),
                                          core_ids=list(range(NCORES)))
    out = np.concatenate([res.results[c]["out_sl"] for c in range(NCORES)],
                         axis=0)
    return np.ascontiguousarray(out[:N_NODES])



# revision 6
# speedup vs baseline: 2.3094x; 2.3094x over previous
"""GAT message-passing kernel for Trainium2 (8 NeuronCores, SPMD).

Target-sharded edge-parallel design (v2). Each core owns NLOC=6272 target
nodes (49 tiles of 128) and all edges targeting them; edges are packed into
128-edge chunks per (tile, lo/hi source arena).

Per-core dataflow:
  phase 0: xT = transposed DMA load of the core's x slice; per-tile score
    matmuls si|sj = xT.T @ [Wi|Wj]; sj AllGather'd (flat 1-D APs) into a
    per-node sjtab in HBM.
  main loop (per 2-tile batch):
    - dma_gather x rows (bf16, 256B) and sj rows (f32, 256B) per edge
    - si / recip broadcast to edges via PE matmuls with host-built
      transposed one-hot (fp8) streamed from HBM
    - e = si+sj; ex = exp(prelu(e)) on Act (single act table)
    - denominator via PE matmuls with host-built one-hot (fp8)
    - alpha = mean_h(ex * recip); scaled one-hot (iota==toff)*alpha built
      on DVE in one tensor_scalar op; output accumulated via PE matmuls
All heavy per-edge operands are bf16/fp8; accumulation stays f32 in PSUM.
"""
import hashlib

import numpy as np
import ml_dtypes

import concourse.mybir as mybir
from concourse import bacc, bass_utils
from concourse.tile import TileContext

P = 128
NCORES = 8
N_NODES = 50000
N_EDGES = 800000
HID = 128
HEADS = 8
NPAD = 50176              # 8 * 6272
NLOC = NPAD // NCORES     # 6272 nodes per core
NT = NLOC // P            # 49 tiles per core
SPLIT = 32768             # lo/hi arena split for int16 dma_gather indices
NEG_SLOPE = 0.01
SJW = 64                  # sjtab row width (f32) -> 256B rows for dma_gather
GMAX = 8                  # slots per dma_gather call (1024 idx HW limit)
TPB = 2                   # tiles per batch
OGRP = 4                  # tiles per output store

F32 = mybir.dt.float32
BF16 = mybir.dt.bfloat16
FP8 = mybir.dt.float8e4

_CACHE = {}
_PREP_CACHE = {}


def _build_program(nclo, nchi, batches):
    nch = [lo + hi for lo, hi in zip(nclo, nchi)]
    nchunks = sum(nch)
    nslot_lo = sum(nclo) * P
    nslot_hi = sum(nchi) * P
    lo_base = np.cumsum([0] + nclo).tolist()
    hi_base = np.cumsum([0] + nchi).tolist()
    ch_base = np.cumsum([0] + nch).tolist()

    nc = bacc.Bacc("TRN2", num_devices=NCORES)

    xpadb = nc.dram_tensor("xpadb", [NPAD, HID], BF16, kind="ExternalInput")
    xsliceb = nc.dram_tensor("xsliceb", [NLOC, HID], BF16, kind="ExternalInput")
    wcatb = nc.dram_tensor("wcatb", [HID, 2 * HEADS], BF16, kind="ExternalInput")
    idxlo = nc.dram_tensor("idxlo", [P, max(nslot_lo // 16, 1)],
                           mybir.dt.int16, kind="ExternalInput")
    idxhi = nc.dram_tensor("idxhi", [P, max(nslot_hi // 16, 1)],
                           mybir.dt.int16, kind="ExternalInput")
    toffin = nc.dram_tensor("toffin", [P, nchunks], F32, kind="ExternalInput")
    ohdram = nc.dram_tensor("ohdram", [P, nchunks * P], FP8,
                            kind="ExternalInput")
    ohtdram = nc.dram_tensor("ohtdram", [P, nchunks * P], FP8,
                             kind="ExternalInput")
    out_sl = nc.dram_tensor("out_sl", [NLOC, HID], F32, kind="ExternalOutput")

    sjtab = nc.dram_tensor("sjtab", [NPAD, SJW], F32, kind="Internal")
    cc_in = nc.dram_tensor("cc_in", [NLOC * HEADS], F32, kind="Internal")
    cc_out = nc.dram_tensor("cc_out", [NPAD * HEADS], F32, kind="Internal",
                            addr_space="Shared")

    iota_d = nc.inline_tensor(
        np.tile(np.arange(P, dtype=ml_dtypes.bfloat16), (P, 1)), name="iotab")

    AF = mybir.ActivationFunctionType
    OP = mybir.AluOpType

    with TileContext(nc) as tc:
        with tc.tile_pool(name="const", bufs=1) as constp:
            iota_sb = constp.tile([P, P], BF16)
            nc.sync.dma_start(out=iota_sb[:], in_=iota_d[:, :])
            wc_sb = constp.tile([HID, 2 * HEADS], BF16)
            nc.sync.dma_start(out=wc_sb[:], in_=wcatb[:, :])
            toff_sb = constp.tile([P, nchunks], F32)
            nc.sync.dma_start(out=toff_sb[:], in_=toffin[:, :])
            ixlo_sb = constp.tile([P, max(nslot_lo // 16, 1)], mybir.dt.int16)
            nc.sync.dma_start(out=ixlo_sb[:], in_=idxlo[:, :])
            ixhi_sb = constp.tile([P, max(nslot_hi // 16, 1)], mybir.dt.int16)
            nc.sync.dma_start(out=ixhi_sb[:], in_=idxhi[:, :])

            # transposed load of this core's x slice: xT[d, n]
            xT_sb = constp.tile([P, NLOC], BF16)
            nc.sync.dma_start(
                out=xT_sb[:],
                in_=xsliceb[:, :].rearrange("n d -> d n"))

            si_sb = constp.tile([P, NT * HEADS], BF16)
            sjf_sb = constp.tile([P, NT * HEADS], F32)

            # ---------- phase 0: score tables ----------
            with tc.tile_pool(name="ph0ps", bufs=2, space="PSUM") as ph0ps:
                half = (NT + 1) // 2
                for g0 in range(0, NT, half):
                    g1 = min(g0 + half, NT)
                    sc_ps = ph0ps.tile([P, half * 2 * HEADS], F32, space="PSUM",
                                       tag="scp")
                    for j in range(g0, g1):
                        o = (j - g0) * 2 * HEADS
                        nc.tensor.matmul(
                            out=sc_ps[:, o:o + 2 * HEADS],
                            lhsT=xT_sb[:, j * P:(j + 1) * P],
                            rhs=wc_sb[:], start=True, stop=True)
                    k = g1 - g0
                    nc.scalar.copy(
                        out=si_sb[:, g0 * HEADS:g1 * HEADS].rearrange(
                            "p (j w) -> p j w", j=k),
                        in_=sc_ps[:, 0:k * 2 * HEADS].rearrange(
                            "p (j w) -> p j w", j=k)[:, :, 0:HEADS])
                    nc.scalar.copy(
                        out=sjf_sb[:, g0 * HEADS:g1 * HEADS].rearrange(
                            "p (j w) -> p j w", j=k),
                        in_=sc_ps[:, 0:k * 2 * HEADS].rearrange(
                            "p (j w) -> p j w", j=k)[:, :, HEADS:2 * HEADS])

            nc.sync.dma_start(
                out=cc_in[:].rearrange("(j p w) -> p j w", p=P, w=HEADS),
                in_=sjf_sb[:].rearrange("p (j w) -> p j w", j=NT))
            nc.gpsimd.collective_compute(
                "AllGather", OP.bypass,
                replica_groups=[list(range(NCORES))],
                ins=[cc_in[:]], outs=[cc_out[:]],
            )
            nc.sync.dma_start(
                out=sjtab[:, 0:HEADS].rearrange("(j p) w -> p j w", p=P),
                in_=cc_out[:].rearrange("(j p w) -> p j w", p=P, w=HEADS))

            with tc.tile_pool(name="gat", bufs=2) as gatp, \
                 tc.tile_pool(name="str", bufs=2) as strp, \
                 tc.tile_pool(name="wk", bufs=4) as wkp, \
                 tc.tile_pool(name="ohsp", bufs=5) as ohsp, \
                 tc.tile_pool(name="recp", bufs=4) as recp, \
                 tc.tile_pool(name="outp", bufs=2) as outp, \
                 tc.tile_pool(name="psA", bufs=2, space="PSUM") as psA, \
                 tc.tile_pool(name="psB", bufs=2, space="PSUM") as psB, \
                 tc.tile_pool(name="psD", bufs=2, space="PSUM") as psD, \
                 tc.tile_pool(name="psO", bufs=2, space="PSUM") as psO:

                def gcalls(dst, table_ap, idx_sb, s0, s1, ew):
                    for g0 in range(0, s1 - s0, GMAX):
                        g1 = min(g0 + GMAX, s1 - s0)
                        nidx = (g1 - g0) * P
                        nc.gpsimd.dma_gather(
                            out_ap=dst[:, g0:g1, :], in_ap=table_ap,
                            idxs_ap=idx_sb[:, (s0 + g0) * 8:(s0 + g1) * 8],
                            num_idxs=nidx, num_idxs_reg=nidx, elem_size=ew)

                obuf = None
                for (t0, t1) in batches:
                    blo0, blo1 = lo_base[t0], lo_base[t1]
                    bhi0, bhi1 = hi_base[t0], hi_base[t1]
                    bch0, bch1 = ch_base[t0], ch_base[t1]
                    bc = bch1 - bch0

                    gx_lo = gatp.tile([P, max(blo1 - blo0, 1), HID], BF16,
                                      tag="gxlo")
                    gs_lo = gatp.tile([P, max(blo1 - blo0, 1), SJW], F32,
                                      tag="gslo")
                    if blo1 > blo0:
                        gcalls(gx_lo[:], xpadb[:, :], ixlo_sb, blo0, blo1, HID)
                        gcalls(gs_lo[:], sjtab[:, :], ixlo_sb, blo0, blo1, SJW)
                    gx_hi = gatp.tile([P, max(bhi1 - bhi0, 1), HID], BF16,
                                      tag="gxhi")
                    gs_hi = gatp.tile([P, max(bhi1 - bhi0, 1), SJW], F32,
                                      tag="gshi")
                    if bhi1 > bhi0:
                        gcalls(gx_hi[:], xpadb[SPLIT:NPAD, :], ixhi_sb,
                               bhi0, bhi1, HID)
                        gcalls(gs_hi[:], sjtab[SPLIT:NPAD, :], ixhi_sb,
                               bhi0, bhi1, SJW)

                    # one-hot streams (SP / Act HWDGE queues)
                    oht_sb = strp.tile([P, bc * P], FP8, tag="oht")
                    nc.sync.dma_start(
                        out=oht_sb[:], in_=ohtdram[:, bch0 * P:bch1 * P])
                    oh_sb = strp.tile([P, bc * P], FP8, tag="oh")
                    nc.scalar.dma_start(
                        out=oh_sb[:], in_=ohdram[:, bch0 * P:bch1 * P])

                    tile_state = []
                    for j in range(t0, t1):
                        ncj = nch[j]
                        nlo_j = nclo[j]
                        ch0 = ch_base[j]
                        co = ch0 - bch0

                        # ---- phase A ----
                        se_ps = psA.tile([P, ncj * HEADS], F32, space="PSUM",
                                         tag="seps")
                        for c in range(ncj):
                            nc.tensor.matmul(
                                out=se_ps[:, c * HEADS:(c + 1) * HEADS],
                                lhsT=oht_sb[:, (co + c) * P:(co + c + 1) * P],
                                rhs=si_sb[:, j * HEADS:(j + 1) * HEADS],
                                start=True, stop=True)
                        e_sb = wkp.tile([P, ncj * HEADS], BF16, tag="esb")
                        if nlo_j:
                            nc.vector.tensor_tensor(
                                out=e_sb[:, 0:nlo_j * HEADS].rearrange(
                                    "p (k w) -> p k w", k=nlo_j),
                                in0=se_ps[:, 0:nlo_j * HEADS].rearrange(
                                    "p (k w) -> p k w", k=nlo_j),
                                in1=gs_lo[:, lo_base[j] - blo0:
                                          lo_base[j] - blo0 + nlo_j, 0:HEADS],
                                op=OP.add)
                        if ncj > nlo_j:
                            nhi_j = ncj - nlo_j
                            nc.vector.tensor_tensor(
                                out=e_sb[:, nlo_j * HEADS:ncj * HEADS].rearrange(
                                    "p (k w) -> p k w", k=nhi_j),
                                in0=se_ps[:, nlo_j * HEADS:ncj * HEADS].rearrange(
                                    "p (k w) -> p k w", k=nhi_j),
                                in1=gs_hi[:, hi_base[j] - bhi0:
                                          hi_base[j] - bhi0 + nhi_j, 0:HEADS],
                                op=OP.add)
                        lr = wkp.tile([P, ncj * HEADS], BF16, tag="lr")
                        nc.scalar.activation(out=lr[:], in_=e_sb[:],
                                             func=AF.Prelu, alpha=NEG_SLOPE)
                        ex = wkp.tile([P, ncj * HEADS], BF16, tag="ex")
                        nc.scalar.activation(out=ex[:], in_=lr[:], func=AF.Exp)

                        den_ps = psD.tile([P, HEADS], F32, space="PSUM",
                                          tag="denps")
                        for c in range(ncj):
                            nc.tensor.matmul(
                                out=den_ps[:],
                                lhsT=oh_sb[:, (co + c) * P:(co + c + 1) * P],
                                rhs=ex[:, c * HEADS:(c + 1) * HEADS],
                                start=(c == 0), stop=(c == ncj - 1))
                        r1 = recp.tile([P, 2 * HEADS], F32, tag="r1")
                        nc.vector.tensor_scalar(
                            out=r1[:, 0:HEADS], in0=den_ps[:], scalar1=1e-30,
                            scalar2=None, op0=OP.max)
                        nc.vector.reciprocal(out=r1[:, HEADS:2 * HEADS],
                                             in_=r1[:, 0:HEADS])
                        rec = recp.tile([P, HEADS], BF16, tag="rec")
                        nc.vector.tensor_scalar(
                            out=rec[:], in0=r1[:, HEADS:2 * HEADS],
                            scalar1=1.0 / HEADS, scalar2=None, op0=OP.mult)
                        tile_state.append((j, ex, rec))

                    for (j, ex, rec) in tile_state:
                        ncj = nch[j]
                        nlo_j = nclo[j]
                        ch0 = ch_base[j]
                        co = ch0 - bch0

                        # ---- phase B ----
                        re_ps = psB.tile([P, ncj * HEADS], F32, space="PSUM",
                                         tag="reps")
                        for c in range(ncj):
                            nc.tensor.matmul(
                                out=re_ps[:, c * HEADS:(c + 1) * HEADS],
                                lhsT=oht_sb[:, (co + c) * P:(co + c + 1) * P],
                                rhs=rec[:], start=True, stop=True)
                        prod = wkp.tile([P, ncj * HEADS], F32, tag="prod")
                        nc.vector.tensor_tensor(
                            out=prod[:], in0=re_ps[:], in1=ex[:], op=OP.mult)
                        alpha = wkp.tile([P, ncj], F32, tag="alpha")
                        nc.vector.reduce_sum(
                            out=alpha[:],
                            in_=prod[:].rearrange("p (k w) -> p k w", k=ncj),
                            axis=mybir.AxisListType.X)

                        out_ps = psO.tile([P, HID], F32, space="PSUM",
                                          tag="outps")
                        for c in range(ncj):
                            ohs = ohsp.tile([P, P], BF16, tag="ohs")
                            nc.vector.tensor_scalar(
                                out=ohs[:], in0=iota_sb[:],
                                scalar1=toff_sb[:, ch0 + c:ch0 + c + 1],
                                scalar2=alpha[:, c:c + 1],
                                op0=OP.is_equal, op1=OP.mult)
                            if c < nlo_j:
                                gx_ap = gx_lo[:, lo_base[j] - blo0 + c, :]
                            else:
                                gx_ap = gx_hi[:, hi_base[j] - bhi0 + (c - nlo_j), :]
                            nc.tensor.matmul(out=out_ps[:], lhsT=ohs[:],
                                             rhs=gx_ap,
                                             start=(c == 0), stop=(c == ncj - 1))
                        g = j % OGRP
                        if g == 0:
                            ng = min(OGRP, NT - j)
                            obuf = outp.tile([P, OGRP * HID], F32, tag="obuf")
                        nc.scalar.copy(out=obuf[:, g * HID:(g + 1) * HID],
                                       in_=out_ps[:])
                        if g == ng - 1:
                            j0 = j - g
                            nc.sync.dma_start(
                                out=out_sl[j0 * P:(j0 + ng) * P, :].rearrange(
                                    "(k p) d -> p k d", p=P),
                                in_=obuf[:, 0:ng * HID].rearrange(
                                    "p (k d) -> p k d", k=ng))

    nc.compile()
    return nc


def _prep(edge_index):
    """Host-side edge layout -> per-core indices, toff, one-hot streams."""
    key = hashlib.sha1(np.ascontiguousarray(edge_index).tobytes()).hexdigest()
    if key in _PREP_CACHE:
        return _PREP_CACHE[key]

    src = edge_index[0].astype(np.int64)
    tgt = edge_index[1].astype(np.int64)
    core = tgt // NLOC
    tile = (tgt % NLOC) // P
    toff = tgt % P
    lo = src < SPLIT

    counts = np.zeros((NCORES, NT, 2), np.int64)
    np.add.at(counts, (core, tile, (~lo).astype(np.int64)), 1)
    nclo = [int(np.ceil(max(counts[:, j, 0].max(), 1) / P)) for j in range(NT)]
    nchi = [int(np.ceil(counts[:, j, 1].max() / P))
            if counts[:, j, 1].max() > 0 else 0 for j in range(NT)]

    nch = [a + b for a, b in zip(nclo, nchi)]
    nchunks = sum(nch)
    nslot_lo = sum(nclo) * P
    nslot_hi = sum(nchi) * P
    lo_base = np.cumsum([0] + nclo)
    hi_base = np.cumsum([0] + nchi)
    ch_base = np.cumsum([0] + nch)

    order = np.lexsort((tile, core))
    src_s, tile_s, toff_s, lo_s, core_s = (
        src[order], tile[order], toff[order], lo[order], core[order])
    cuts = np.searchsorted(core_s, np.arange(NCORES + 1))

    def wrap16(a):
        if len(a) == 0:
            return np.zeros((P, 1), np.int16)
        w = a.reshape(-1, 16).T
        return np.tile(w, (8, 1)).astype(np.int16)

    tgrid = np.arange(P, dtype=np.int64)[None, :]
    per_core = []
    for c in range(NCORES):
        s, e = cuts[c], cuts[c + 1]
        csrc, ctile, ctoff, clo = (src_s[s:e], tile_s[s:e], toff_s[s:e],
                                   lo_s[s:e])
        ilo = np.zeros(nslot_lo, np.int16)
        ihi = np.zeros(nslot_hi, np.int16)
        tof = np.full(nchunks * P, 999.0, np.float32)
        tcuts = np.searchsorted(ctile, np.arange(NT + 1))
        for j in range(NT):
            js, je = tcuts[j], tcuts[j + 1]
            jsrc, jtoff, jlo = csrc[js:je], ctoff[js:je], clo[js:je]
            sel = jlo
            n = int(sel.sum())
            ilo[lo_base[j] * P:lo_base[j] * P + n] = jsrc[sel].astype(np.int16)
            cb = ch_base[j] * P
            tof[cb:cb + n] = jtoff[sel]
            sel = ~jlo
            m = int(sel.sum())
            ihi[hi_base[j] * P:hi_base[j] * P + m] = \
                (jsrc[sel] - SPLIT).astype(np.int16)
            cb2 = (ch_base[j] + nclo[j]) * P
            tof[cb2:cb2 + m] = jtoff[sel]

        tof2 = tof.reshape(nchunks, P)
        onehot = (tof2[:, :, None] == tgrid[None, :, :])   # [c, p, t]
        ohdram = np.ascontiguousarray(
            onehot.transpose(1, 0, 2).reshape(P, nchunks * P)).astype(
                ml_dtypes.float8_e4m3)
        ohtdram = np.ascontiguousarray(
            onehot.transpose(2, 0, 1).reshape(P, nchunks * P)).astype(
                ml_dtypes.float8_e4m3)

        per_core.append({
            "idxlo": wrap16(ilo),
            "idxhi": wrap16(ihi),
            "toffin": np.ascontiguousarray(tof2.T).astype(np.float32),
            "ohdram": ohdram,
            "ohtdram": ohtdram,
        })
    res = (nclo, nchi, per_core)
    _PREP_CACHE.clear()
    _PREP_CACHE[key] = res
    return res


def _in_maps(inputs, per_core):
    xpadb = np.zeros((NPAD, HID), ml_dtypes.bfloat16)
    xpadb[:N_NODES] = inputs["x"].astype(ml_dtypes.bfloat16)
    wcatb = np.concatenate([np.asarray(inputs["Wi"]).T,
                            np.asarray(inputs["Wj"]).T],
                           axis=1).astype(ml_dtypes.bfloat16)
    maps = []
    for c in range(NCORES):
        m = dict(per_core[c])
        m["xpadb"] = xpadb
        m["xsliceb"] = np.ascontiguousarray(xpadb[c * NLOC:(c + 1) * NLOC])
        m["wcatb"] = wcatb
        maps.append(m)
    return maps


def kernel(x, Wi, Wj, edge_index):
    inputs = {"x": np.asarray(x, np.float32),
              "Wi": np.asarray(Wi, np.float32),
              "Wj": np.asarray(Wj, np.float32)}
    edge_index = np.asarray(edge_index)

    nclo, nchi, per_core = _prep(edge_index)
    key = (tuple(nclo), tuple(nchi))
    if key not in _CACHE:
        batches = [(t, min(t + TPB, NT)) for t in range(0, NT, TPB)]
        _CACHE.clear()
        _CACHE[key] = _build_program(nclo, nchi, batches)
    nc = _CACHE[key]

    res = bass_utils.run_bass_kernel_spmd(nc, _in_maps(inputs, per_core),
                                          core_ids=list(range(NCORES)))
    out = np.concatenate([res.results[c]["out_sl"] for c in range(NCORES)],
                         axis=0)
    return np.ascontiguousarray(out[:N_NODES])
